# revision 57
# baseline (speedup 1.0000x reference)
"""Trainium2 Bass kernel for CrossAttention (sparse_attention variant).

Reference computation (shapes hardcoded):
  x [2, 1024, 1024], context [2, 4, 1024, 1024], doc_similarities [2, 4]
  q = x @ Wq, kv = ctx @ Wkv (k|v), dots = q k^T / sqrt(d) + doc_bias,
  attn = softmax(dots over all 4096 doc tokens), out = (attn @ v) @ Wout + bout

Sharding: 8 cores = 2 batches x 4 head-pairs.  Core c: batch c//4, heads
{2*(c%4), 2*(c%4)+1}.  Each core computes a [1024, 1024] partial of the
output projection (its heads' rows of Wout); host sums 4 partials per batch.

Implementation notes (all matmul inputs bf16: 1 cycle/row at any free size,
rel err ~6e-3 vs the 2e-2 gate; f32 PSUM accumulation throughout):
  - Single software pipeline over 8 j-chunks of 512 doc tokens: chunk c's
    program projects K/V(c), computes QK+exp for chunk c-1 and E@V for
    chunk c-2.  This overlaps the ScalarE exp stream (~78us total) with
    projection matmuls so PE never waits on softmax.
  - K^T [hd, j] from lhsT=Wk tiles; V directly in natural [j, hd] layout
    from lhsT=ctx^T slices (no PE transposes).
  - Softmax denominator: exp tiles are accumulated elementwise into bf16
    accumulators on DVE (ic=0) and GpSimd (ic=1) -- engines that are
    otherwise idle -- then one tiny ones-vector matmul per (head, ic)
    reduces the 128 j-lanes.  This removes the per-j-tile ones-matmuls
    (64k PE cycles) of the naive approach.  Per-lane bf16 rounding errors
    average out across the 128-lane final reduction (~0.1% on dn).
  - PSUM budget (8 banks): proj ring 3, st ring 3, y ring 2.  E@V
    accumulates per chunk into the y ring and is drained to an f32 SBUF
    accumulator by DVE adds.
  - Normalization via reciprocal + PE row-broadcast, then the partial
    output projection (rows of Wout for this core's heads).
"""

import numpy as np
import ml_dtypes
from contextlib import ExitStack

import concourse.bass as bass
import concourse.mybir as mybir
import concourse.tile as tile
from concourse import bacc
from concourse import bass_utils

# Problem constants
B, N, M, CN, D = 2, 1024, 4, 1024, 1024
H = 8          # total heads
HPC = 2        # heads per core
NCORES = 8
HD = D // H    # 128
J = M * CN     # 4096
KT = D // 128  # 8 contraction k-tiles
IC = N // 512  # 2 i-chunks of queries
JC = J // 512  # 8 j-chunks (pipeline granularity)
JT = J // 128  # 32 j-tiles (attention granularity)
SCALE = float(D ** -0.5)

BF = mybir.dt.bfloat16
FR = mybir.dt.float32r
F32 = mybir.dt.float32
NPBF = ml_dtypes.bfloat16

_NC_CACHE = {}
LAST_RESULT = None


def _build_module(reps=1):
    nc = bacc.Bacc(
        "TRN2",
        target_bir_lowering=False,
        debug=False,
        num_devices=NCORES,
    )

    # xT/ctxT arrive pre-tiled on the host: [128, KT, cols], so a single
    # DMA can carry multiple 128-row contraction tiles.
    xT = nc.dram_tensor("xT", [128, KT, N], BF, kind="ExternalInput").ap()
    ctxT = nc.dram_tensor("ctxT", [128, KT, J], BF, kind="ExternalInput").ap()
    wq = nc.dram_tensor("wq", [128, KT * HPC * HD], BF, kind="ExternalInput").ap()
    wk = nc.dram_tensor("wk", [128, KT * HPC * HD], BF, kind="ExternalInput").ap()
    wv = nc.dram_tensor("wv", [128, KT * HPC * HD], BF, kind="ExternalInput").ap()
    wout = nc.dram_tensor("wout", [128, HPC * D], BF, kind="ExternalInput").ap()
    docb = nc.dram_tensor("docb", [128, JT], F32, kind="ExternalInput").ap()
    # bf16 output halves writeback DMA (all transfers share one 360GB/s
    # path); host converts back.  Costs ~0.4% extra element error vs the
    # 2e-2 gate.
    outp = nc.dram_tensor("outp", [N, D], BF, kind="ExternalOutput").ap()

    EXP = mybir.ActivationFunctionType.Exp

    with tile.TileContext(nc) as tc:
        with ExitStack() as ctx:
          wpool = ctx.enter_context(tc.tile_pool(name="wpool", bufs=1))
          big = ctx.enter_context(tc.tile_pool(name="big", bufs=1))
          stream = ctx.enter_context(tc.tile_pool(name="stream", bufs=4))
          epool = ctx.enter_context(tc.tile_pool(name="epool", bufs=4))
          pp = ctx.enter_context(tc.tile_pool(name="pp", bufs=2, space="PSUM"))
          for _rep in range(reps):
              # ---- constants ----
              ones_col = wpool.tile([128, 1], BF, name="ones_col")
              nc.vector.memset(ones_col[:, :], 1.0)
              # fp32r constants built in f32 then copied (memset can't emit FR)
              ones_row_f = wpool.tile([1, 128], F32, name="ones_row_f")
              nc.vector.memset(ones_row_f[:, :], 1.0)
              ones_row = wpool.tile([1, 128], FR, name="ones_row")
              nc.vector.tensor_copy(ones_row[:, :], ones_row_f[:, :])

              docb_sb = wpool.tile([128, JT], F32, name="docb_sb")

              # ---- weights / activations in SBUF ----
              wq_sb = wpool.tile([128, KT, HPC * HD], BF, name="wq_sb")
              wk_sb = wpool.tile([128, KT, HPC * HD], BF, name="wk_sb")
              wv_sb = wpool.tile([128, KT, HPC * HD], BF, name="wv_sb")
              wout_sb = wpool.tile([128, HPC, D], BF, name="wout_sb")
              xt_sb = wpool.tile([128, KT, N], BF, name="xt_sb")

              qT_sb = big.tile([128, HPC, N], BF, name="qT_sb")     # q^T [hd, h, i]
              kT_sb = big.tile([128, HPC, J], BF, name="kT_sb")     # k^T [hd, h, j]
              vn_sb = big.tile([128, JT * HPC * HD], BF, name="vn_sb")  # v [j, (h hd)]
              yn_sb = big.tile([128, HPC, N], BF, name="yn_sb")     # Ynorm^T [hd, h, i]
              # y accumulator, ping-pong per (h, ic): [pp][128, h, ic, 512]
              y_acc = [
                  big.tile([128, HPC, IC, 512], F32, name=f"y_acc{p}")
                  for p in range(2)
              ]
              # dn accumulators, ping-pong per (h, ic)
              dn_acc = [
                  big.tile([128, HPC, IC, 512], BF, name=f"dn_acc{p}")
                  for p in range(2)
              ]
              rs_sb = big.tile([128, HPC, N], F32, name="rs_sb")
              recip = big.tile([1, HPC, N], FR, name="recip")

              ct_tiles = {}  # (chunk, kt) -> (pair tile, index)

              def dma_ct_pair(c, p, eng=None):
                  # one DMA covers two adjacent contraction tiles
                  t = stream.tile([128, 2, 512], BF, name="ct", tag="ct", bufs=12)
                  (eng or nc.sync).dma_start(
                      out=t[:, :, :],
                      in_=ctxT[:, 2 * p:2 * p + 2, c * 512:(c + 1) * 512],
                  )
                  ct_tiles[(c, 2 * p)] = (t, 0)
                  ct_tiles[(c, 2 * p + 1)] = (t, 1)

              def dma_ct(c, eng=None):
                  for p in range(KT // 2):
                      dma_ct_pair(c, p, eng)

              def ct_rhs(c, kt):
                  t, i = ct_tiles[(c, kt)]
                  return t[:, i, :]

              def ct_lhs(c, kt, sl):
                  t, i = ct_tiles[(c, kt)]
                  return t[:, i, sl * 128:(sl + 1) * 128]

              # ---- input DMAs needed up front.  The first two DMAs are the
              # minimal payload for the first Q matmul (wq k-tile 0, first
              # half of x^T k-tile 0) so PE starts ~2us earlier. ----
              # HWDGE processes roughly one DMA per ~0.66us globally, so the
              # pipeline fill is paced by DMA count and issue order: the
              # Q-projection inputs go first, chunk-0 context after.
              nc.sync.dma_start(out=wq_sb[:, 0, :], in_=wq[:, 0:256])
              nc.sync.dma_start(out=xt_sb[:, 0, :], in_=xT[:, 0, :])
              nc.sync.dma_start(out=wq_sb[:, 1:, :], in_=wq[:, 256:])
              nc.sync.dma_start(out=xt_sb[:, 1, :], in_=xT[:, 1, :])
              for p in range(1, KT // 2):
                  nc.sync.dma_start(
                      out=xt_sb[:, 2 * p:2 * p + 2, :],
                      in_=xT[:, 2 * p:2 * p + 2, :],
                  )
              nc.sync.dma_start(out=wk_sb[:, :, :], in_=wk[:, :])
              dma_ct(0)
              nc.sync.dma_start(out=docb_sb[:, :], in_=docb[:, :])
              nc.sync.dma_start(out=wv_sb[:, :, :], in_=wv[:, :])
              dma_ct(1)

              # ---- Q projection (kt-outer so the first matmul only waits
              # on the first xt DMA; 4 concurrent PSUM groups) ----
              qps = {}
              for ic in range(IC):
                  for h in range(HPC):
                      tag = "proj" if (ic, h) != (1, 1) else "st"
                      qps[(ic, h)] = pp.tile([128, 512], F32, name="qp",
                                             tag=tag, bufs=3)
              for kt in range(KT):
                  for ic in range(IC):
                      for h in range(HPC):
                          nc.tensor.matmul(
                              qps[(ic, h)][:, :],
                              lhsT=wq_sb[:, kt, h * HD:(h + 1) * HD],
                              rhs=xt_sb[:, kt, ic * 512:(ic + 1) * 512],
                              start=(kt == 0),
                              stop=(kt == KT - 1),
                          )
              for ic in range(IC):
                  for h in range(HPC):
                      nc.vector.tensor_copy(
                          qT_sb[:, h, ic * 512:(ic + 1) * 512],
                          qps[(ic, h)][:, :],
                      )

              # ---- fused projection + attention pipeline over j-chunks ----
              # Chunk c program: [K proj 16 mms] [kT evicts] then 8 blocks of
              # [4 V mms, qk pair (lag 0), ev pair (lag 1)].  QK for chunk c
              # runs right after its own K eviction; E@V consumes chunk c-1's
              # exp tiles, giving ScalarE a full chunk of slack.
              et_tiles = {}       # (j, h, ic) -> SBUF bf16 exp tile
              dn_count = {}       # (h, ic) -> adds so far (ping-pong index)
              y_count = {}        # (h, ic) -> chunk-partials folded so far

              def emit_qk_pair(j, h, dn_defer):
                  # QK (2 mms) + exp (2 ScalarE) for one (j-tile, head).
                  sts = []
                  for ic in range(IC):
                      st = pp.tile([128, 512], F32, name="st", tag="st", bufs=3)
                      nc.tensor.matmul(
                          st[:, :],
                          lhsT=kT_sb[:, h, j * 128:(j + 1) * 128],
                          rhs=qT_sb[:, h, ic * 512:(ic + 1) * 512],
                          start=True,
                          stop=True,
                      )
                      sts.append(st)
                  for ic in range(IC):
                      et = epool.tile([128, 512], BF, name="et",
                                      tag="et", bufs=40)
                      nc.scalar.activation(
                          et[:, :],
                          sts[ic][:, :],
                          EXP,
                          bias=docb_sb[:, j:j + 1],
                          scale=SCALE,
                      )
                      et_tiles[(j, h, ic)] = et
                      if ic == 0:
                          # DVE dn chain: deferred past the y folds so the
                          # fold adds aren't queued behind it on DVE
                          dn_defer.append((j, h, ic))
                      else:
                          emit_dn_add(j, h, ic)  # GpSimd chain

              def emit_dn_add(j, h, ic):
                  et = et_tiles[(j, h, ic)]
                  cnt = dn_count.get((h, ic), 0)
                  src = dn_acc[cnt % 2][:, h, ic, :]
                  dst = dn_acc[(cnt + 1) % 2][:, h, ic, :]
                  # ic1 chains run mostly on GpSimd; every 4th add goes to
                  # DVE (except in the last chunk) so the slower GpSimd
                  # doesn't accumulate a backlog that gates the epilogue
                  if ic == 0:
                      eng = nc.vector
                  elif j % 4 == 3 and j < (JC - 1) * 4:
                      eng = nc.vector
                  else:
                      eng = nc.gpsimd
                  if cnt == 0:
                      eng.tensor_copy(dst, et[:, :])
                  else:
                      eng.tensor_add(dst, src, et[:, :])
                  dn_count[(h, ic)] = cnt + 1

              def ev_mm(yp, j, h, ic, jo):
                  nc.tensor.matmul(
                      yp[:, :],
                      lhsT=vn_sb[:, j * 256 + h * HD:j * 256 + (h + 1) * HD],
                      rhs=et_tiles[(j, h, ic)][:, :],
                      start=(jo == 0),
                      stop=(jo == 3),
                  )

              def fold_y(yp, h, ic):
                  cnt = y_count.get((h, ic), 0)
                  dst = y_acc[(cnt + 1) % 2][:, h, ic, :]
                  if cnt == 0:
                      nc.vector.tensor_copy(dst, yp[:, :])
                  else:
                      nc.vector.tensor_add(dst, y_acc[cnt % 2][:, h, ic, :], yp[:, :])
                  y_count[(h, ic)] = cnt + 1

              def ev_half(ec, half, yps, tag="yp"):
                  # allocate the two E@V PSUM groups for this half
                  for ic in range(IC):
                      yps[ic] = pp.tile([128, 512], F32, name="yp",
                                        tag=tag, bufs=(3 if tag == "proj" else 2))

              def ev_half_close(ec, half, yps, dn_defer):
                  for ic in range(IC):
                      fold_y(yps[ic], half, ic)
                  for (j, h, ic) in dn_defer:
                      emit_dn_add(j, h, ic)
                  dn_defer.clear()
                  for jo in range(4):
                      for ic in range(IC):
                          del et_tiles[(ec * 4 + jo, half, ic)]

              for c in range(JC):
                  ec = c - 1 if c >= 1 else None
                  # K projection: kT[hd, j-chunk] per head
                  kp = [
                      pp.tile([128, 512], F32, name=f"kp{h}", tag="proj", bufs=3)
                      for h in range(HPC)
                  ]
                  for kt in range(KT):
                      for h in range(HPC):
                          nc.tensor.matmul(
                              kp[h][:, :],
                              lhsT=wk_sb[:, kt, h * HD:(h + 1) * HD],
                              rhs=ct_rhs(c, kt),
                              start=(kt == 0),
                              stop=(kt == KT - 1),
                          )
                  for h in range(HPC):
                      nc.vector.tensor_copy(
                          kT_sb[:, h, c * 512:(c + 1) * 512], kp[h][:, :]
                      )
                  if c == 0:
                      nc.sync.dma_start(out=wout_sb[:, :, :], in_=wout[:, :])
                  if c + 2 <= JC - 1:
                      dma_ct(c + 2)
                  # mixed section: V projection + QK(c) + E@V(c-1)
                  dn_defer = []
                  vps = {}
                  yps = {}
                  for blk in range(8):
                      pair = blk // 4
                      if blk % 4 == 0:
                          vps[pair] = pp.tile([128, 512], F32, name="vp",
                                              tag="proj", bufs=3)
                          if ec is not None:
                              ev_half(ec, pair, yps)
                      # 4 V mms, sl-sequential: one j-slice finishes all its
                      # contraction tiles before the next starts.  The two
                      # slices share one PSUM bank, and start_tensor_calc
                      # zeroes the whole 2KB bank region -- interleaving the
                      # two accumulation groups would corrupt the first.
                      sloff = (blk % 4) // 2
                      sl = 2 * pair + sloff
                      for kt in range(4 * (blk % 2), 4 * (blk % 2) + 4):
                          nc.tensor.matmul(
                              vps[pair][:, sloff * 256:sloff * 256 + 256],
                              lhsT=ct_lhs(c, kt, sl),
                              rhs=wv_sb[:, kt, :],
                              start=(kt == 0),
                              stop=(kt == KT - 1),
                          )
                      # QK pair for this chunk (lag 0)
                      emit_qk_pair(c * 4 + blk // 2, blk % 2, dn_defer)
                      # E@V pair for previous chunk (lag 1)
                      if ec is not None:
                          for ic in range(IC):
                              ev_mm(yps[ic], ec * 4 + blk % 4, pair, ic, blk % 4)
                      if blk % 4 == 3:
                          # pair 0: folds first (they gate the next EV half);
                          # pair 1: vn eviction first (it gates the next
                          # chunk's K projection PSUM reuse)
                          jt = c * 4 + 2 * pair
                          if pair == 1:
                              nc.vector.tensor_copy(
                                  vn_sb[:, jt * 256:(jt + 2) * 256],
                                  vps[pair][:, :],
                              )
                          if ec is not None:
                              ev_half_close(ec, pair, yps, dn_defer)
                          else:
                              for item in dn_defer:
                                  emit_dn_add(*item)
                              dn_defer.clear()
                          if pair == 0:
                              nc.vector.tensor_copy(
                                  vn_sb[:, jt * 256:(jt + 2) * 256],
                                  vps[pair][:, :],
                              )
                  for kt in range(KT):
                      del ct_tiles[(c, kt)]

              # ---- tail: E@V for chunk 7 with the normalization chains
              # (denominator reduce -> reciprocal -> broadcast -> scale row)
              # overlapped into the E@V sections; the first half of the
              # output projection starts while the GpSimd dn chains (ic=1)
              # and second epilogue are still draining ----
              def epi_pre(ic):
                  # everything up to (not including) the yn multiplies
                  dnps = []
                  for h in range(HPC):
                      cnt = dn_count[(h, ic)]
                      dnp = pp.tile([1, 512], F32, name="dnp", tag="st", bufs=3)
                      nc.tensor.matmul(
                          dnp[:, :],
                          lhsT=ones_col[:, :],
                          rhs=dn_acc[cnt % 2][:, h, ic, :],
                          start=True,
                          stop=True,
                      )
                      dnps.append(dnp)
                  for h in range(HPC):
                      with nc.allow_low_precision(
                          reason="float32r output is 32-bit, same as float32"
                      ):
                          nc.vector.reciprocal(
                              recip[:, h, ic * 512:(ic + 1) * 512], dnps[h][:, :]
                          )
                  for h in range(HPC):
                      bc = pp.tile([128, 512], F32, name="bc", tag="yp", bufs=2)
                      nc.tensor.matmul(
                          bc[:, :],
                          lhsT=ones_row[:, :],
                          rhs=recip[:, h, ic * 512:(ic + 1) * 512],
                          start=True,
                          stop=True,
                      )
                      nc.scalar.copy(rs_sb[:, h, ic * 512:(ic + 1) * 512], bc[:, :])

              def yn_muls(ic):
                  # ic0 on DVE (free at that point); ic1 on GpSimd so it
                  # isn't queued behind the first output rows' DVE evictions
                  eng = nc.vector if ic == 0 else nc.gpsimd
                  for h in range(HPC):
                      ycnt = y_count[(h, ic)]
                      eng.tensor_mul(
                          yn_sb[:, h, ic * 512:(ic + 1) * 512],
                          y_acc[ycnt % 2][:, h, ic, :],
                          rs_sb[:, h, ic * 512:(ic + 1) * 512],
                      )

              def out_proj(its):
                  # h-inner-oc order reuses each yn stationary tile twice;
                  # PSUM rings alternate proj/st tags; evictions alternate
                  # DVE/ScalarE; merged [128, 1024] DMAs on the idle SP
                  # queue, with the last row block split across two queues.
                  last = its[-1]
                  for it in its:
                      ops = []
                      for oc in range(D // 512):
                          ops.append(pp.tile([128, 512], F32, name="op",
                                             tag=("proj" if it % 2 == 0 else "st"),
                                             bufs=3))
                      for h in range(HPC):
                          for oc in range(D // 512):
                              nc.tensor.matmul(
                                  ops[oc][:, :],
                                  lhsT=yn_sb[:, h, it * 128:(it + 1) * 128],
                                  rhs=wout_sb[:, h, oc * 512:(oc + 1) * 512],
                                  start=(h == 0),
                                  stop=(h == HPC - 1),
                              )
                      ot = stream.tile([128, 1024], BF, name="ot", tag="ot",
                                       bufs=4)
                      nc.vector.tensor_copy(ot[:, 0:512], ops[0][:, :])
                      nc.scalar.copy(ot[:, 512:1024], ops[1][:, :])
                      if it == last:
                          nc.sync.dma_start(
                              out=outp[it * 128:(it + 1) * 128, 0:512],
                              in_=ot[:, 0:512],
                          )
                          nc.scalar.dma_start(
                              out=outp[it * 128:(it + 1) * 128, 512:1024],
                              in_=ot[:, 512:1024],
                          )
                      else:
                          nc.sync.dma_start(
                              out=outp[it * 128:(it + 1) * 128, :],
                              in_=ot[:, :],
                          )

              ec = JC - 1
              dn_defer = []
              for half in range(HPC):
                  yps = {}
                  # the tail E@V groups borrow the proj PSUM ring (idle by
                  # now) so they don't WAR-wait on chunk 7's y folds
                  ev_half(ec, half, yps, tag="proj")
                  for jo in range(4):
                      for ic in range(IC):
                          ev_mm(yps[ic], ec * 4 + jo, half, ic, jo)
                  ev_half_close(ec, half, yps, dn_defer)
                  if half == 0:
                      # ic0 dn chains (DVE) are already complete; run the
                      # ic0 normalization chain under the h1 E@V section
                      epi_pre(0)
              # its 0-3 need only the ic0 yn halves; the ic1 chain drains
              # while they run
              yn_muls(0)
              epi_pre(1)
              yn_muls(1)
              out_proj(list(range(0, 8)))

    nc.compile()
    return nc


def get_nc(reps=1):
    if reps not in _NC_CACHE:
        _NC_CACHE[reps] = _build_module(reps)
    return _NC_CACHE[reps]


def make_in_maps(inputs):
    x = np.asarray(inputs["x"], dtype=np.float32)
    context = np.asarray(inputs["context"], dtype=np.float32)
    doc = np.asarray(inputs["doc_similarities"], dtype=np.float32)
    cmask = np.asarray(inputs["context_mask"])
    Wq = np.asarray(inputs["Wq"], dtype=np.float32)
    Wkv = np.asarray(inputs["Wkv"], dtype=np.float32)
    beta = float(np.asarray(inputs["beta"]))
    Wout = np.asarray(inputs["Wout"], dtype=np.float32)

    per_batch = []
    for b in range(B):
        # pre-tiled [128, KT, cols]: partition line q of k-tile kt holds
        # row kt*128+q of the transposed activation matrix
        xT = np.ascontiguousarray(
            x[b].T.reshape(KT, 128, N).transpose(1, 0, 2)
        ).astype(NPBF)
        ctxT = np.ascontiguousarray(
            context[b].reshape(J, D).T.reshape(KT, 128, J).transpose(1, 0, 2)
        ).astype(NPBF)
        bias = np.repeat(doc[b], CN) * beta
        bias = np.where(cmask[b].reshape(J), bias, -1e30).astype(np.float32)
        docb = np.ascontiguousarray(bias.reshape(JT, 128).T)  # [128, JT]
        per_batch.append((xT, ctxT, docb))

    in_maps = []
    for c in range(NCORES):
        b = c // 4
        h0 = (c % 4) * HPC
        xT, ctxT, docb = per_batch[b]

        def pack_kxc(w):
            # [D, C] -> [128, KT*C]: tile rows so each partition line is contiguous
            cc = w.shape[1]
            return np.ascontiguousarray(
                w.reshape(KT, 128, cc).transpose(1, 0, 2).reshape(128, KT * cc)
            ).astype(NPBF)

        wout_c = Wout[h0 * HD:(h0 + HPC) * HD, :]
        in_maps.append({
            "xT": xT,
            "ctxT": ctxT,
            "wq": pack_kxc(Wq[:, h0 * HD:(h0 + HPC) * HD]),
            "wk": pack_kxc(Wkv[:, h0 * HD:(h0 + HPC) * HD]),
            "wv": pack_kxc(Wkv[:, D + h0 * HD:D + (h0 + HPC) * HD]),
            "wout": np.ascontiguousarray(
                wout_c.reshape(HPC, 128, D).transpose(1, 0, 2).reshape(128, HPC * D)
            ).astype(NPBF),
            "docb": docb,
        })
    return in_maps


def kernel(**inputs):
    global LAST_RESULT
    nc = get_nc()
    in_maps = make_in_maps(inputs)
    res = bass_utils.run_bass_kernel_spmd(
        nc, in_maps, core_ids=list(range(NCORES))
    )
    LAST_RESULT = res
    out = np.zeros((B, N, D), dtype=np.float32)
    for c in range(NCORES):
        out[c // 4] += res.results[c]["outp"].astype(np.float32)
    out += np.asarray(inputs["bout"], dtype=np.float32)
    return out
